# revision 50
# baseline (speedup 1.0000x reference)
"""Memory-efficient multi-head attention on 8 TRN2 NeuronCores.

Problem (hardcoded): B=2, L=2048, D=1024, H=16, HD=64.
  out = softmax((X_q Wq + bq)(X_k Wk + bk)^T / sqrt(HD)) (X_v Wv + bv) Wo + bo

Sharding: 8 cores = 2 batches x 4 head-groups (4 heads each).
Each core gets the full (transposed) activations of its batch plus its
head-group's slice of Wq/Wk/Wv (columns) and Wo (rows), and produces a
partial pre-bias output out^T [D, L].  Host sums the 4 partials of each
batch and adds bo.  All matmul work is exact - no duplicated FLOPs.

Device kernel layout choices:
  - Activations are fed pre-transposed (X^T [D, L]) so the contraction dim
    (D) lands on SBUF partitions for every projection matmul.
  - q,k projections produce qT/kT [256, L] (head channels on partitions):
    scores are then computed TRANSPOSED, sT[k_pos, q_pos] = k @ q^T, which
    makes the softmax denominator a partition-dim sum that we fold into the
    attn@v matmul via an appended ones-column on v (output row 64 = denom).
  - v projection produces v in natural [L, 65*4] layout (65 = 64 channels
    + ones column) to serve as the stationary operand of attn@v.
  - exp runs on ScalarE with the 1/sqrt(HD) scale fused into the activation
    and NO max-subtraction: scores are ~N(0,1) here, max over the full
    attention matrix is ~5.5, exp(5.5) is tiny vs fp32 range, and softmax
    is mathematically shift-invariant so the result matches the reference.
  - All matmuls use float32r (E8M11; full-rate fp32 streaming mode on the
    TRN2 PE - regular fp32 runs at 1/4 rate).  Operands are rounded fp32r
    end-to-end: host-side rounding for DMA-fed tensors, engine writeback
    rounding for on-chip producers (the BIR verifier enforces this).
  - The softmax reciprocal row is partition-broadcast on the otherwise-idle
    GpSimd engine.
  - Emission order hand-interleaves projections, attention waves, and the
    out-projection so ScalarE (the busiest engine, ~138us of exp) starts
    ~34us in and PSUM tag slots hand off without cross-phase stalls.
"""

import numpy as np

import concourse.mybir as mybir
import concourse.tile as tile
from concourse import bacc
from concourse.bass_utils import run_bass_kernel_spmd

F32 = mybir.dt.float32
F32R = mybir.dt.float32r


def build_attention_core(L=2048, D=1024, H_LOC=4, HD=64, exp_chunk=1024):
    """Build the single-core kernel (SPMD across 8 cores).

    Per-core I/O:
      xqT,xkT,xvT [D, L]   : batch activations, transposed
      wq,wk,wv    [D, JC]  : weight column slices (JC = H_LOC*HD)
      wo          [JC, D]  : weight row slice
      bq,bk,bv    [1, JC]  : bias slices
      outT        [D, L]   : partial output, transposed, pre-bo
    """
    JC = H_LOC * HD                   # local head channels (256)
    assert JC % 128 == 0 and HD == 64
    NJT = JC // 128                   # j-tiles (2)
    NDT = D // 128                    # contraction tiles (8)
    NLT = L // 128                    # l/k-position tiles (16)
    PC = min(512, L)                  # projection N-chunk
    NPC = L // PC
    QC = min(exp_chunk, L)            # attention q-chunk
    NQC = L // QC
    CS = min(512, QC)                 # matmul N-chunk inside a q-chunk
    NCS = QC // CS

    nc = bacc.Bacc("TRN2", target_bir_lowering=False, debug=False, num_devices=8)

    xqT = nc.dram_tensor("xqT", [D, L], F32R, kind="ExternalInput").ap()
    xkT = nc.dram_tensor("xkT", [D, L], F32R, kind="ExternalInput").ap()
    xvT = nc.dram_tensor("xvT", [D, L], F32R, kind="ExternalInput").ap()
    wq = nc.dram_tensor("wq", [D, JC], F32R, kind="ExternalInput").ap()
    wk = nc.dram_tensor("wk", [D, JC], F32R, kind="ExternalInput").ap()
    wv = nc.dram_tensor("wv", [D, JC], F32R, kind="ExternalInput").ap()
    wo = nc.dram_tensor("wo", [JC, D], F32R, kind="ExternalInput").ap()
    bq = nc.dram_tensor("bq", [1, JC], F32R, kind="ExternalInput").ap()
    bk = nc.dram_tensor("bk", [1, JC], F32R, kind="ExternalInput").ap()
    bv = nc.dram_tensor("bv", [1, JC], F32R, kind="ExternalInput").ap()
    outT = nc.dram_tensor("outT", [D, L], F32, kind="ExternalOutput").ap()

    from contextlib import ExitStack

    SPLIT = NLT // 2 if NQC > 1 else NLT  # kt waves split around input half 1
    XC = min(512, L)                      # x stream tile width
    NXC = L // XC                         # x tiles per row of d-tiles

    with (
        tile.TileContext(nc) as tc,
        ExitStack() as stack,
        nc.allow_low_precision(reason="fp32r (E8M11) operands for full-rate PE"),
    ):
        # All pools are created up front: phases share PSUM/SBUF tag slots in
        # emission order instead of overlapping released address ranges
        # (which would add serializing WAR deps between phases).
        consts = stack.enter_context(tc.tile_pool(name="consts", bufs=1))
        wpool = stack.enter_context(tc.tile_pool(name="wpool", bufs=1))
        prod = stack.enter_context(tc.tile_pool(name="prod", bufs=1))
        xt_pool = stack.enter_context(tc.tile_pool(name="xt", bufs=2 * NDT + 4 if L < 1024 else 18))
        att_pool = stack.enter_context(tc.tile_pool(name="att", bufs=5))
        rden_pool = stack.enter_context(tc.tile_pool(name="rden", bufs=1))
        oc_pool = stack.enter_context(tc.tile_pool(name="oc", bufs=4))
        # PSUM: st0/st1 [128,QC] (2 banks each, short-lived: projections,
        # scores, out-projection) + ot0/ot1 [HD+1,QC] (2 banks each,
        # long-lived attn@v accumulators, also v-proj in half 0) = 8 banks.
        psum = stack.enter_context(tc.tile_pool(name="psum", bufs=1, space="PSUM"))

        ones_f32 = consts.tile([1, PC], F32)
        nc.vector.memset(ones_f32, 1.0)
        ones_row = consts.tile([1, PC], F32R)
        nc.vector.tensor_copy(out=ones_row, in_=ones_f32)

        wq_sb = wpool.tile([128, NDT, JC], F32R, tag="wq")
        wk_sb = wpool.tile([128, NDT, JC], F32R, tag="wk")
        wv_sb = wpool.tile([128, NDT, JC], F32R, tag="wv")
        wo_sb = wpool.tile([128, NJT, D], F32R, tag="wo")
        bq_sb = wpool.tile([1, JC], F32R, tag="bq")
        bk_sb = wpool.tile([1, JC], F32R, tag="bk")
        bv_sb = wpool.tile([1, JC], F32R, tag="bv")
        # k/q weights lead; wv is emitted before the v stream and wo before
        # the first out-projection so the x streams start ASAP.
        nc.sync.dma_start(out=wk_sb, in_=wk.rearrange("(t p) j -> p t j", p=128))
        nc.sync.dma_start(out=bk_sb, in_=bk)
        nc.sync.dma_start(out=wq_sb, in_=wq.rearrange("(t p) j -> p t j", p=128))
        nc.sync.dma_start(out=bq_sb, in_=bq)

        qT_sb = prod.tile([128, NJT, L], F32R, tag="qT")    # [j%128, jt, l]
        kT_sb = prod.tile([128, NJT, L], F32R, tag="kT")
        v_sb = prod.tile([128, NLT, H_LOC, HD + 1], F32R, tag="v")  # ones col
        onorm_sb = prod.tile([128, NJT, L], F32R, tag="onorm")

        vones_f32 = consts.tile([128, NLT * H_LOC], F32)
        nc.vector.memset(vones_f32, 1.0)
        nc.vector.tensor_copy(
            out=v_sb[:, :, :, HD : HD + 1].rearrange("p a b c -> p (a b c)"),
            in_=vones_f32,
        )

        # ---- emission helpers -------------------------------------------
        def load_x(x_dram, l_lo, l_hi):
            """Stream x^T tiles for columns [l_lo, l_hi): tiles[c][dt]."""
            tiles = []
            for c in range(l_lo // XC, l_hi // XC):
                col = []
                for dt in range(NDT):
                    x_t = xt_pool.tile([128, XC], F32R, tag="xt", name="xt")
                    nc.sync.dma_start(
                        out=x_t,
                        in_=x_dram[dt * 128 : dt * 128 + 128,
                                   c * XC : c * XC + XC],
                    )
                    col.append(x_t)
                tiles.append(col)
            return tiles

        def proj_kq(w_sb, b_sb, dst, x_tiles, l_lo, tags=None):
            """dst[j, l] = sum_d W[d, j] xT[d, l] + b[j]  (bias via ones row)
            over columns [l_lo, l_lo + len(x_tiles)*XC)."""
            span = len(x_tiles) * XC
            for jt in range(NJT):
                tag = tags[jt] if tags else f"st{jt}"
                ps = psum.tile([128, span], F32, tag=tag, name="psa")
                for c, col in enumerate(x_tiles):
                    for dt in range(NDT):
                        nc.tensor.matmul(
                            ps[:, c * XC : c * XC + XC],
                            w_sb[:, dt, jt * 128 : jt * 128 + 128],
                            col[dt],
                            start=(dt == 0),
                            stop=False,
                        )
                    nc.tensor.matmul(
                        ps[:, c * XC : c * XC + XC],
                        b_sb[:, jt * 128 : jt * 128 + 128],
                        ones_row[:, 0:XC],
                        start=False,
                        stop=True,
                    )
                    # chunked copy-out: consumers of this slice start sooner
                    nc.vector.tensor_copy(
                        out=dst[:, jt, l_lo + c * XC : l_lo + c * XC + XC],
                        in_=ps[:, c * XC : c * XC + XC],
                    )

        def proj_v_lt(glt, x_tiles, l_lo, tag):
            """v[l, j] = sum_d xT[d, l] W[d, j] + b[j] for one 128-row tile."""
            ps = psum.tile([128, JC], F32, tag=tag, name="psv")
            off = glt * 128 - l_lo
            col = x_tiles[off // XC]
            off %= XC
            for dt in range(NDT):
                nc.tensor.matmul(
                    ps,
                    col[dt][:, off : off + 128],
                    wv_sb[:, dt, :],
                    start=(dt == 0),
                    stop=False,
                )
            nc.tensor.matmul(ps, ones_row[:, 0:128], bv_sb, start=False, stop=True)
            nc.vector.tensor_copy(
                out=v_sb[:, glt, :, 0:HD],
                in_=ps.rearrange("p (h d) -> p h d", h=H_LOC),
            )

        def score_exp_kt(pair, qc, kt):
            """Scores + exp for both heads of a pair at one kt.
            The heads' K=64 score matmuls are adjacent so they run on
            disjoint PE row groups concurrently; the pair of exps covers the
            next kt's score-matmul latency.  Returns the exp tiles."""
            jt = pair
            heads = (2 * pair, 2 * pair + 1)
            sts = {h: psum.tile([128, QC], F32, tag=f"st{h % 2}",
                                name=f"st{h % 2}")
                   for h in heads}
            for c in range(NCS):
                for h in heads:
                    hp = (h % 2) * HD
                    nc.tensor.matmul(
                        sts[h][:, c * CS : c * CS + CS],
                        kT_sb[hp : hp + HD, jt, kt * 128 : kt * 128 + 128],
                        qT_sb[hp : hp + HD, jt,
                              qc * QC + c * CS : qc * QC + c * CS + CS],
                        start=True,
                        stop=True,
                    )
            ats = {}
            for h in heads:
                at = att_pool.tile([128, QC], F32R, tag=f"at{h % 2}",
                                   name=f"at{h % 2}")
                nc.scalar.activation(
                    out=at, in_=sts[h],
                    func=mybir.ActivationFunctionType.Exp,
                    scale=float(1.0 / np.sqrt(HD)),
                )
                ats[h] = at
            return ats

        def av_kt(pair, kt, ats, ots):
            for h in (2 * pair, 2 * pair + 1):
                for c in range(NCS):
                    nc.tensor.matmul(
                        ots[h][:, c * CS : c * CS + CS],
                        v_sb[:, kt, h, :],
                        ats[h][:, c * CS : c * CS + CS],
                        start=(kt == 0),
                        stop=(kt == NLT - 1),
                    )

        def attn_kt(pair, qc, kt, ots):
            av_kt(pair, kt, score_exp_kt(pair, qc, kt), ots)

        def epilogue(pair, qc, ots):
            """onorm[ch, q] = OT[ch, q] * recip(OT[HD, q]); the reciprocal
            row is partition-broadcast on the (otherwise idle) GpSimd."""
            jt = pair
            for h in (2 * pair, 2 * pair + 1):
                hp = (h % 2) * HD
                rden = rden_pool.tile([1, QC], F32, tag="rden", name="rden")
                nc.vector.reciprocal(out=rden, in_=ots[h][HD : HD + 1, :])
                rbc = rden_pool.tile([HD, QC], F32, tag="rbc", name="rbc")
                nc.gpsimd.partition_broadcast(rbc, rden)
                nc.vector.tensor_mul(
                    out=onorm_sb[hp : hp + HD, jt, qc * QC : qc * QC + QC],
                    in0=ots[h][0:HD, :],
                    in1=rbc,
                )

        def make_ots(pair):
            return {h: psum.tile([HD + 1, QC], F32, tag=f"ot{h % 2}",
                                 name=f"ot{h % 2}")
                    for h in (2 * pair, 2 * pair + 1)}

        def outproj_mt(qc, mt, tag, copy_eng):
            """outT[dp, l] = sum_j wo[j, dp] onorm[j, l], one 128-row tile."""
            ps = psum.tile([128, QC], F32, tag=tag, name="psc")
            for c in range(NCS):
                for jt in range(NJT):
                    nc.tensor.matmul(
                        ps[:, c * CS : c * CS + CS],
                        wo_sb[:, jt, mt * 128 : mt * 128 + 128],
                        onorm_sb[:, jt,
                                 qc * QC + c * CS : qc * QC + c * CS + CS],
                        start=(jt == 0),
                        stop=(jt == NJT - 1),
                    )
            ob = oc_pool.tile([128, QC], F32, tag="oc", name="oc")
            if copy_eng == "scalar":
                nc.scalar.copy(out=ob, in_=ps)
            else:
                nc.vector.tensor_copy(out=ob, in_=ps)
            dma_eng = nc.gpsimd if mt % 2 else nc.sync
            dma_eng.dma_start(
                out=outT[mt * 128 : mt * 128 + 128, qc * QC : qc * QC + QC],
                in_=ob,
            )

        def outproj(qc, tags=("st0", "st1"), alt_copy=False):
            for mt in range(NDT):
                outproj_mt(qc, mt, tags[mt % len(tags)],
                           "scalar" if alt_copy and mt % 2 else "vector")

        # ---- emission schedule ------------------------------------------
        # Half 0 of k, q, v streams (v projections use the ot PSUM tags so
        # the first score matmuls aren't slot-blocked), then attention
        # (pair 0, qc 0) on kt<SPLIT while half 1 streams in.  In the
        # kt>=SPLIT wave each step first projects its own v tile, aligning
        # slot hand-offs with the data deps.  q half 1 (only needed for
        # qc>0) and the out-projections are emitted where PE has slack.
        H1 = SPLIT * 128  # l where half 1 starts
        # interleave k/q quarter loads: the first scores need only the c0
        # quarters of each, so their 6MB prefix leads the DMA queue
        xk = load_x(xkT, 0, XC)
        xq = load_x(xqT, 0, XC)
        xk += load_x(xkT, XC, H1)
        xq += load_x(xqT, XC, min(QC, L))
        proj_kq(wk_sb, bk_sb, kT_sb, xk, 0, tags=("st0", "ot0"))
        proj_kq(wq_sb, bq_sb, qT_sb, xq, 0, tags=("st1", "ot1"))
        nc.sync.dma_start(out=wv_sb, in_=wv.rearrange("(t p) j -> p t j", p=128))
        nc.sync.dma_start(out=bv_sb, in_=bv)
        xv = load_x(xvT, 0, H1)
        for lt in range(SPLIT):
            proj_v_lt(lt, xv, 0, tag=f"ot{lt % 2}")
        ots0 = make_ots(0)
        xk1 = load_x(xkT, H1, L) if SPLIT < NLT else None
        w0_ats = []
        for kt in range(SPLIT):  # exps don't wait on the v stream
            w0_ats.append(score_exp_kt(0, 0, kt))
            if xk1 is not None and kt == SPLIT - 3:
                proj_kq(wk_sb, bk_sb, kT_sb, xk1, H1)
        for kt in range(SPLIT):
            av_kt(0, kt, w0_ats[kt], ots0)
        if SPLIT < NLT:
            xv1 = load_x(xvT, H1, L)
            w1_ats = {}
            for kt in range(SPLIT, NLT):
                w1_ats[kt] = score_exp_kt(0, 0, kt)
                if kt >= SPLIT + 2:  # v-projs+avs trail the exps
                    proj_v_lt(kt - 2, xv1, H1, tag=f"st{kt % 2}")
                    av_kt(0, kt - 2, w1_ats.pop(kt - 2), ots0)
            for kt in (NLT - 2, NLT - 1):
                proj_v_lt(kt, xv1, H1, tag=f"st{kt % 2}")
                av_kt(0, kt, w1_ats.pop(kt), ots0)
        epilogue(0, 0, ots0)
        if QC < L:
            xq1 = load_x(xqT, QC, L)
        nc.sync.dma_start(out=wo_sb, in_=wo.rearrange("(t p) j -> p t j", p=128))
        for pair in range(1, H_LOC // 2):
            ots = make_ots(pair)
            for kt in range(NLT):
                attn_kt(pair, 0, kt, ots)
            if QC < L:
                proj_kq(wq_sb, bq_sb, qT_sb, xq1, QC)
            epilogue(pair, 0, ots)
        last_qc = NQC - 1
        for qc in range(1, NQC):
            for pair in range(H_LOC // 2):
                ots = make_ots(pair)
                for kt in range(NLT):
                    attn_kt(pair, qc, kt, ots)
                    # out-projection of the previous chunk rides the PE slack
                    # of the last pair's ACT-bound kt loop
                    if qc == last_qc and pair == H_LOC // 2 - 1 and kt % 2 == 1:
                        mt = kt // 2
                        if mt < NDT:
                            outproj_mt(qc - 1, mt, f"st{mt % 2}", "vector")
                epilogue(pair, qc, ots)
        # tail: deep pipeline over all four PSUM tags, copies on both engines
        outproj(last_qc if NQC > 1 else 0,
                tags=("st0", "st1", "ot0", "ot1"), alt_copy=True)

    nc.compile()
    return nc


_NC_CACHE = {}


def _get_nc():
    if "nc" not in _NC_CACHE:
        _NC_CACHE["nc"] = build_attention_core()
    return _NC_CACHE["nc"]


def round_fp32r(x):
    """Round fp32 to the fp32r (E8M11) grid, nearest-even - matches HW."""
    u = np.ascontiguousarray(x, np.float32).view(np.uint32)
    u = (u + np.uint32(0x7FF) + ((u >> np.uint32(12)) & np.uint32(1))) & np.uint32(0xFFFFF000)
    return u.view(np.float32)


def shard_inputs(query, key_, value, Wq, bq, Wk, bk, Wv, bv, Wo, bo,
                 B=2, H=16, H_LOC=4, HD=64):
    """Host-side sharding: core c -> (batch c//4, head-group c%4)."""
    groups = H // H_LOC
    xT = [round_fp32r(np.ascontiguousarray(np.asarray(x, np.float32).transpose(0, 2, 1)))
          for x in (query, key_, value)]
    in_maps = []
    for c in range(B * groups):
        b, g = divmod(c, groups)
        js = slice(g * H_LOC * HD, (g + 1) * H_LOC * HD)
        in_maps.append({
            "xqT": xT[0][b], "xkT": xT[1][b], "xvT": xT[2][b],
            "wq": round_fp32r(np.ascontiguousarray(np.asarray(Wq, np.float32)[:, js])),
            "wk": round_fp32r(np.ascontiguousarray(np.asarray(Wk, np.float32)[:, js])),
            "wv": round_fp32r(np.ascontiguousarray(np.asarray(Wv, np.float32)[:, js])),
            "wo": round_fp32r(np.ascontiguousarray(np.asarray(Wo, np.float32)[js, :])),
            "bq": round_fp32r(np.asarray(bq, np.float32)[None, js]),
            "bk": round_fp32r(np.asarray(bk, np.float32)[None, js]),
            "bv": round_fp32r(np.asarray(bv, np.float32)[None, js]),
        })
    return in_maps


def kernel(query, key_, value, Wq, bq, Wk, bk, Wv, bv, Wo, bo):
    B, L, D = 2, 2048, 1024
    groups = 4
    nc = _get_nc()
    in_maps = shard_inputs(query, key_, value, Wq, bq, Wk, bk, Wv, bv, Wo, bo)
    res = run_bass_kernel_spmd(nc, in_maps, list(range(8))).results
    out = np.empty((B, L, D), np.float32)
    bo = np.asarray(bo, np.float32)
    for b in range(B):
        acc = res[b * groups]["outT"].astype(np.float32)
        for g in range(1, groups):
            acc = acc + res[b * groups + g]["outT"]
        out[b] = acc.T + bo
    return out


# revision 61
# speedup vs baseline: 1.0029x; 1.0029x over previous
"""Memory-efficient multi-head attention on 8 TRN2 NeuronCores.

Problem (hardcoded): B=2, L=2048, D=1024, H=16, HD=64.
  out = softmax((X_q Wq + bq)(X_k Wk + bk)^T / sqrt(HD)) (X_v Wv + bv) Wo + bo

Sharding: 8 cores = 2 batches x 4 head-groups (4 heads each).
Each core gets the full (transposed) activations of its batch plus its
head-group's slice of Wq/Wk/Wv (columns) and Wo (rows), and produces a
partial pre-bias output out^T [D, L].  Host sums the 4 partials of each
batch and adds bo.  All matmul work is exact - no duplicated FLOPs.

Device kernel layout choices:
  - Activations are fed pre-transposed (X^T [D, L]) so the contraction dim
    (D) lands on SBUF partitions for every projection matmul.
  - q,k projections produce qT/kT [256, L] (head channels on partitions):
    scores are then computed TRANSPOSED, sT[k_pos, q_pos] = k @ q^T, which
    makes the softmax denominator a partition-dim sum that we fold into the
    attn@v matmul via an appended ones-column on v (output row 64 = denom).
  - v projection produces v in natural [L, 65*4] layout (65 = 64 channels
    + ones column) to serve as the stationary operand of attn@v.
  - exp runs on ScalarE with the 1/sqrt(HD) scale fused into the activation
    and NO max-subtraction: scores are ~N(0,1) here, max over the full
    attention matrix is ~5.5, exp(5.5) is tiny vs fp32 range, and softmax
    is mathematically shift-invariant so the result matches the reference.
  - All matmuls use float32r (E8M11; full-rate fp32 streaming mode on the
    TRN2 PE - regular fp32 runs at 1/4 rate).  Operands are rounded fp32r
    end-to-end: host-side rounding for DMA-fed tensors, engine writeback
    rounding for on-chip producers (the BIR verifier enforces this).
  - The softmax reciprocal row is partition-broadcast on the otherwise-idle
    GpSimd engine.
  - Emission order hand-interleaves projections, attention waves, and the
    out-projection so ScalarE (the busiest engine, ~138us of exp) starts
    ~34us in and PSUM tag slots hand off without cross-phase stalls.
"""

import numpy as np

import concourse.mybir as mybir
import concourse.tile as tile
from concourse import bacc
from concourse.bass_utils import run_bass_kernel_spmd

F32 = mybir.dt.float32
F32R = mybir.dt.float32r


def build_attention_core(L=2048, D=1024, H_LOC=4, HD=64, exp_chunk=1024):
    """Build the single-core kernel (SPMD across 8 cores).

    Per-core I/O:
      xqT,xkT,xvT [D, L]   : batch activations, transposed
      wq,wk,wv    [D, JC]  : weight column slices (JC = H_LOC*HD)
      wo          [JC, D]  : weight row slice
      bq,bk,bv    [1, JC]  : bias slices
      outT        [D, L]   : partial output, transposed, pre-bo
    """
    JC = H_LOC * HD                   # local head channels (256)
    assert JC % 128 == 0 and HD == 64
    NJT = JC // 128                   # j-tiles (2)
    NDT = D // 128                    # contraction tiles (8)
    NLT = L // 128                    # l/k-position tiles (16)
    PC = min(512, L)                  # projection N-chunk
    NPC = L // PC
    QC = min(exp_chunk, L)            # attention q-chunk
    NQC = L // QC
    CS = min(512, QC)                 # matmul N-chunk inside a q-chunk
    NCS = QC // CS

    nc = bacc.Bacc("TRN2", target_bir_lowering=False, debug=False, num_devices=8)

    xqT = nc.dram_tensor("xqT", [D, L], F32R, kind="ExternalInput").ap()
    xkT = nc.dram_tensor("xkT", [D, L], F32R, kind="ExternalInput").ap()
    xvT = nc.dram_tensor("xvT", [D, L], F32R, kind="ExternalInput").ap()
    wq = nc.dram_tensor("wq", [D, JC], F32R, kind="ExternalInput").ap()
    wk = nc.dram_tensor("wk", [D, JC], F32R, kind="ExternalInput").ap()
    wv = nc.dram_tensor("wv", [D, JC], F32R, kind="ExternalInput").ap()
    wo = nc.dram_tensor("wo", [JC, D], F32R, kind="ExternalInput").ap()
    bq = nc.dram_tensor("bq", [1, JC], F32R, kind="ExternalInput").ap()
    bk = nc.dram_tensor("bk", [1, JC], F32R, kind="ExternalInput").ap()
    bv = nc.dram_tensor("bv", [1, JC], F32R, kind="ExternalInput").ap()
    outT = nc.dram_tensor("outT", [D, L], F32, kind="ExternalOutput").ap()

    from contextlib import ExitStack

    SPLIT = NLT // 2 if NQC > 1 else NLT  # kt waves split around input half 1
    XC = min(512, L)                      # x stream tile width
    NXC = L // XC                         # x tiles per row of d-tiles

    with (
        tile.TileContext(nc) as tc,
        ExitStack() as stack,
        nc.allow_low_precision(reason="fp32r (E8M11) operands for full-rate PE"),
    ):
        # All pools are created up front: phases share PSUM/SBUF tag slots in
        # emission order instead of overlapping released address ranges
        # (which would add serializing WAR deps between phases).
        consts = stack.enter_context(tc.tile_pool(name="consts", bufs=1))
        wpool = stack.enter_context(tc.tile_pool(name="wpool", bufs=1))
        prod = stack.enter_context(tc.tile_pool(name="prod", bufs=1))
        xt_pool = stack.enter_context(tc.tile_pool(name="xt", bufs=2 * NDT + 4 if L < 1024 else 16))
        att_pool = stack.enter_context(tc.tile_pool(name="att", bufs=6))
        rden_pool = stack.enter_context(tc.tile_pool(name="rden", bufs=1))
        oc_pool = stack.enter_context(tc.tile_pool(name="oc", bufs=4))
        # PSUM: st0/st1 [128,QC] (2 banks each, short-lived: projections,
        # scores, out-projection) + ot0/ot1 [HD+1,QC] (2 banks each,
        # long-lived attn@v accumulators, also v-proj in half 0) = 8 banks.
        psum = stack.enter_context(tc.tile_pool(name="psum", bufs=1, space="PSUM"))

        ones_f32 = consts.tile([1, PC], F32)
        nc.vector.memset(ones_f32, 1.0)
        ones_row = consts.tile([1, PC], F32R)
        nc.vector.tensor_copy(out=ones_row, in_=ones_f32)

        wq_sb = wpool.tile([128, NDT, JC], F32R, tag="wq")
        wk_sb = wpool.tile([128, NDT, JC], F32R, tag="wk")
        wv_sb = wpool.tile([128, NDT, JC], F32R, tag="wv")
        wo_sb = wpool.tile([128, NJT, D], F32R, tag="wo")
        bq_sb = wpool.tile([1, JC], F32R, tag="bq")
        bk_sb = wpool.tile([1, JC], F32R, tag="bk")
        bv_sb = wpool.tile([1, JC], F32R, tag="bv")
        # k/q weights lead; wv is emitted before the v stream and wo before
        # the first out-projection so the x streams start ASAP.
        nc.sync.dma_start(out=wk_sb, in_=wk.rearrange("(t p) j -> p t j", p=128))
        nc.sync.dma_start(out=bk_sb, in_=bk)
        nc.sync.dma_start(out=wq_sb, in_=wq.rearrange("(t p) j -> p t j", p=128))
        nc.sync.dma_start(out=bq_sb, in_=bq)

        qT_sb = prod.tile([128, NJT, L], F32R, tag="qT")    # [j%128, jt, l]
        kT_sb = prod.tile([128, NJT, L], F32R, tag="kT")
        v_sb = prod.tile([128, NLT, H_LOC, HD + 1], F32R, tag="v")  # ones col
        onorm_sb = prod.tile([128, NJT, L], F32R, tag="onorm")

        vones_f32 = consts.tile([128, NLT * H_LOC], F32)
        nc.vector.memset(vones_f32, 1.0)
        nc.vector.tensor_copy(
            out=v_sb[:, :, :, HD : HD + 1].rearrange("p a b c -> p (a b c)"),
            in_=vones_f32,
        )

        # ---- emission helpers -------------------------------------------
        def load_x(x_dram, l_lo, l_hi):
            """Stream x^T tiles for columns [l_lo, l_hi): tiles[c][dt]."""
            tiles = []
            for c in range(l_lo // XC, l_hi // XC):
                col = []
                for dt in range(NDT):
                    x_t = xt_pool.tile([128, XC], F32R, tag="xt", name="xt")
                    nc.sync.dma_start(
                        out=x_t,
                        in_=x_dram[dt * 128 : dt * 128 + 128,
                                   c * XC : c * XC + XC],
                    )
                    col.append(x_t)
                tiles.append(col)
            return tiles

        def proj_kq(w_sb, b_sb, dst, x_tiles, l_lo, tags=None):
            """dst[j, l] = sum_d W[d, j] xT[d, l] + b[j]  (bias via ones row)
            over columns [l_lo, l_lo + len(x_tiles)*XC)."""
            span = len(x_tiles) * XC
            for jt in range(NJT):
                tag = tags[jt] if tags else f"st{jt}"
                ps = psum.tile([128, span], F32, tag=tag, name="psa")
                for c, col in enumerate(x_tiles):
                    for dt in range(NDT):
                        nc.tensor.matmul(
                            ps[:, c * XC : c * XC + XC],
                            w_sb[:, dt, jt * 128 : jt * 128 + 128],
                            col[dt],
                            start=(dt == 0),
                            stop=False,
                        )
                    nc.tensor.matmul(
                        ps[:, c * XC : c * XC + XC],
                        b_sb[:, jt * 128 : jt * 128 + 128],
                        ones_row[:, 0:XC],
                        start=False,
                        stop=True,
                    )
                    # chunked copy-out: consumers of this slice start sooner
                    nc.vector.tensor_copy(
                        out=dst[:, jt, l_lo + c * XC : l_lo + c * XC + XC],
                        in_=ps[:, c * XC : c * XC + XC],
                    )

        def proj_v_lt(glt, x_tiles, l_lo, tag):
            """v[l, j] = sum_d xT[d, l] W[d, j] + b[j] for one 128-row tile."""
            ps = psum.tile([128, JC], F32, tag=tag, name="psv")
            off = glt * 128 - l_lo
            col = x_tiles[off // XC]
            off %= XC
            for dt in range(NDT):
                nc.tensor.matmul(
                    ps,
                    col[dt][:, off : off + 128],
                    wv_sb[:, dt, :],
                    start=(dt == 0),
                    stop=False,
                )
            nc.tensor.matmul(ps, ones_row[:, 0:128], bv_sb, start=False, stop=True)
            nc.vector.tensor_copy(
                out=v_sb[:, glt, :, 0:HD],
                in_=ps.rearrange("p (h d) -> p h d", h=H_LOC),
            )

        def score_exp_kt(pair, qc, kt):
            """Scores + exp for both heads of a pair at one kt.
            The heads' K=64 score matmuls are adjacent so they run on
            disjoint PE row groups concurrently; the pair of exps covers the
            next kt's score-matmul latency.  Returns the exp tiles."""
            jt = pair
            heads = (2 * pair, 2 * pair + 1)
            sts = {h: psum.tile([128, QC], F32, tag=f"st{h % 2}",
                                name=f"st{h % 2}")
                   for h in heads}
            for c in range(NCS):
                for h in heads:
                    hp = (h % 2) * HD
                    nc.tensor.matmul(
                        sts[h][:, c * CS : c * CS + CS],
                        kT_sb[hp : hp + HD, jt, kt * 128 : kt * 128 + 128],
                        qT_sb[hp : hp + HD, jt,
                              qc * QC + c * CS : qc * QC + c * CS + CS],
                        start=True,
                        stop=True,
                    )
            ats = {}
            for h in heads:
                at = att_pool.tile([128, QC], F32R, tag=f"at{h % 2}",
                                   name=f"at{h % 2}")
                nc.scalar.activation(
                    out=at, in_=sts[h],
                    func=mybir.ActivationFunctionType.Exp,
                    scale=float(1.0 / np.sqrt(HD)),
                )
                ats[h] = at
            return ats

        def av_kt(pair, kt, ats, ots):
            for h in (2 * pair, 2 * pair + 1):
                for c in range(NCS):
                    nc.tensor.matmul(
                        ots[h][:, c * CS : c * CS + CS],
                        v_sb[:, kt, h, :],
                        ats[h][:, c * CS : c * CS + CS],
                        start=(kt == 0),
                        stop=(kt == NLT - 1),
                    )

        def attn_kt(pair, qc, kt, ots):
            av_kt(pair, kt, score_exp_kt(pair, qc, kt), ots)

        def epilogue(pair, qc, ots):
            """onorm[ch, q] = OT[ch, q] * recip(OT[HD, q]); the reciprocal
            row is partition-broadcast on the (otherwise idle) GpSimd."""
            jt = pair
            for h in (2 * pair, 2 * pair + 1):
                hp = (h % 2) * HD
                rden = rden_pool.tile([1, QC], F32, tag="rden", name="rden")
                nc.vector.reciprocal(out=rden, in_=ots[h][HD : HD + 1, :])
                rbc = rden_pool.tile([HD, QC], F32, tag="rbc", name="rbc")
                nc.gpsimd.partition_broadcast(rbc, rden)
                nc.vector.tensor_mul(
                    out=onorm_sb[hp : hp + HD, jt, qc * QC : qc * QC + QC],
                    in0=ots[h][0:HD, :],
                    in1=rbc,
                )

        def make_ots(pair):
            return {h: psum.tile([HD + 1, QC], F32, tag=f"ot{h % 2}",
                                 name=f"ot{h % 2}")
                    for h in (2 * pair, 2 * pair + 1)}

        def outproj_mt(qc, mt, tag, copy_eng):
            """outT[dp, l] = sum_j wo[j, dp] onorm[j, l], one 128-row tile."""
            ps = psum.tile([128, QC], F32, tag=tag, name="psc")
            for c in range(NCS):
                for jt in range(NJT):
                    nc.tensor.matmul(
                        ps[:, c * CS : c * CS + CS],
                        wo_sb[:, jt, mt * 128 : mt * 128 + 128],
                        onorm_sb[:, jt,
                                 qc * QC + c * CS : qc * QC + c * CS + CS],
                        start=(jt == 0),
                        stop=(jt == NJT - 1),
                    )
            ob = oc_pool.tile([128, QC], F32, tag="oc", name="oc")
            if copy_eng == "scalar":
                nc.scalar.copy(out=ob, in_=ps)
            else:
                nc.vector.tensor_copy(out=ob, in_=ps)
            dma_eng = nc.gpsimd if mt % 2 else nc.sync
            dma_eng.dma_start(
                out=outT[mt * 128 : mt * 128 + 128, qc * QC : qc * QC + QC],
                in_=ob,
            )

        def outproj(qc, tags=("st0", "st1"), alt_copy=False):
            for mt in range(NDT):
                outproj_mt(qc, mt, tags[mt % len(tags)],
                           "scalar" if alt_copy and mt % 2 else "vector")

        # ---- emission schedule ------------------------------------------
        # Half 0 of k, q, v streams (v projections use the ot PSUM tags so
        # the first score matmuls aren't slot-blocked), then attention
        # (pair 0, qc 0) on kt<SPLIT while half 1 streams in.  In the
        # kt>=SPLIT wave each step first projects its own v tile, aligning
        # slot hand-offs with the data deps.  q half 1 (only needed for
        # qc>0) and the out-projections are emitted where PE has slack.
        H1 = SPLIT * 128  # l where half 1 starts
        # interleave k/q quarter loads: the first scores need only the c0
        # quarters of each, so their 6MB prefix leads the DMA queue
        xk = load_x(xkT, 0, XC)
        xq = load_x(xqT, 0, XC)
        xk += load_x(xkT, XC, H1)
        xq += load_x(xqT, XC, min(QC, L))
        proj_kq(wk_sb, bk_sb, kT_sb, xk, 0, tags=("st0", "ot0"))
        proj_kq(wq_sb, bq_sb, qT_sb, xq, 0, tags=("st1", "ot1"))
        nc.sync.dma_start(out=wv_sb, in_=wv.rearrange("(t p) j -> p t j", p=128))
        nc.sync.dma_start(out=bv_sb, in_=bv)
        xv = load_x(xvT, 0, H1)
        for lt in range(SPLIT):
            proj_v_lt(lt, xv, 0, tag=f"ot{lt % 2}")
        ots0 = make_ots(0)
        xk1 = load_x(xkT, H1, L) if SPLIT < NLT else None
        w0_ats = []
        for kt in range(SPLIT):  # exps don't wait on the v stream
            w0_ats.append(score_exp_kt(0, 0, kt))
            if xk1 is not None and kt == SPLIT - 3:
                proj_kq(wk_sb, bk_sb, kT_sb, xk1, H1)
        for kt in range(SPLIT):
            av_kt(0, kt, w0_ats[kt], ots0)
        if SPLIT < NLT:
            xv1 = load_x(xvT, H1, L)
            w1_ats = {}
            for kt in range(SPLIT, NLT):
                w1_ats[kt] = score_exp_kt(0, 0, kt)
                if kt >= SPLIT + 2:  # v-projs+avs trail the exps
                    proj_v_lt(kt - 2, xv1, H1, tag=f"st{kt % 2}")
                    av_kt(0, kt - 2, w1_ats.pop(kt - 2), ots0)
            for kt in (NLT - 2, NLT - 1):
                proj_v_lt(kt, xv1, H1, tag=f"st{kt % 2}")
                av_kt(0, kt, w1_ats.pop(kt), ots0)
        epilogue(0, 0, ots0)
        if QC < L:
            xq1 = load_x(xqT, QC, L)
        nc.sync.dma_start(out=wo_sb, in_=wo.rearrange("(t p) j -> p t j", p=128))
        for pair in range(1, H_LOC // 2):
            ots = make_ots(pair)
            for kt in range(NLT):
                attn_kt(pair, 0, kt, ots)
            if QC < L:
                proj_kq(wq_sb, bq_sb, qT_sb, xq1, QC)
            epilogue(pair, 0, ots)
        last_qc = NQC - 1
        for qc in range(1, NQC):
            for pair in range(H_LOC // 2):
                ots = make_ots(pair)
                for kt in range(NLT):
                    attn_kt(pair, qc, kt, ots)
                    # out-projection of the previous chunk rides the PE slack
                    # of the last pair's ACT-bound kt loop
                    if qc == last_qc and pair == H_LOC // 2 - 1 and kt % 2 == 1:
                        mt = kt // 2
                        if mt < NDT:
                            outproj_mt(qc - 1, mt, f"st{mt % 2}", "vector")
                epilogue(pair, qc, ots)
        # tail: deep pipeline over all four PSUM tags, copies on both engines
        outproj(last_qc if NQC > 1 else 0,
                tags=("st0", "st1", "ot0", "ot1"), alt_copy=True)

    nc.compile()
    return nc


_NC_CACHE = {}


def _get_nc():
    if "nc" not in _NC_CACHE:
        _NC_CACHE["nc"] = build_attention_core()
    return _NC_CACHE["nc"]


def round_fp32r(x):
    """Round fp32 to the fp32r (E8M11) grid, nearest-even - matches HW."""
    u = np.ascontiguousarray(x, np.float32).view(np.uint32)
    u = (u + np.uint32(0x7FF) + ((u >> np.uint32(12)) & np.uint32(1))) & np.uint32(0xFFFFF000)
    return u.view(np.float32)


def shard_inputs(query, key_, value, Wq, bq, Wk, bk, Wv, bv, Wo, bo,
                 B=2, H=16, H_LOC=4, HD=64):
    """Host-side sharding: core c -> (batch c//4, head-group c%4)."""
    groups = H // H_LOC
    xT = [round_fp32r(np.ascontiguousarray(np.asarray(x, np.float32).transpose(0, 2, 1)))
          for x in (query, key_, value)]
    in_maps = []
    for c in range(B * groups):
        b, g = divmod(c, groups)
        js = slice(g * H_LOC * HD, (g + 1) * H_LOC * HD)
        in_maps.append({
            "xqT": xT[0][b], "xkT": xT[1][b], "xvT": xT[2][b],
            "wq": round_fp32r(np.ascontiguousarray(np.asarray(Wq, np.float32)[:, js])),
            "wk": round_fp32r(np.ascontiguousarray(np.asarray(Wk, np.float32)[:, js])),
            "wv": round_fp32r(np.ascontiguousarray(np.asarray(Wv, np.float32)[:, js])),
            "wo": round_fp32r(np.ascontiguousarray(np.asarray(Wo, np.float32)[js, :])),
            "bq": round_fp32r(np.asarray(bq, np.float32)[None, js]),
            "bk": round_fp32r(np.asarray(bk, np.float32)[None, js]),
            "bv": round_fp32r(np.asarray(bv, np.float32)[None, js]),
        })
    return in_maps


def kernel(query, key_, value, Wq, bq, Wk, bk, Wv, bv, Wo, bo):
    B, L, D = 2, 2048, 1024
    groups = 4
    nc = _get_nc()
    in_maps = shard_inputs(query, key_, value, Wq, bq, Wk, bk, Wv, bv, Wo, bo)
    res = run_bass_kernel_spmd(nc, in_maps, list(range(8))).results
    out = np.empty((B, L, D), np.float32)
    bo = np.asarray(bo, np.float32)
    for b in range(B):
        acc = res[b * groups]["outT"].astype(np.float32)
        for g in range(1, groups):
            acc = acc + res[b * groups + g]["outT"]
        out[b] = acc.T + bo
    return out
